# revision 1
# baseline (speedup 1.0000x reference)
"""DistMult edge scoring on 8 Trainium2 NeuronCores.

score[e] = sum_d node_emb[src[e], d] * rel_emb[e, d] * node_emb[dst[e], d]

Strategy (data-parallel over edges, node table replicated per core):
  - Each of the 8 cores gets the full node_emb table in its HBM plus a
    1/8 shard of the edges (rel rows + src/dst indices).
  - The node table is split into 4 blocks of 25000 rows. Each core's
    edges are bucketed host-side into 16 groups by (src_block,
    dst_block), so within a group both gathers address a <32768-row
    window of the table and the fast GPSIMD dma_gather ucode (int16
    local indices, ~0.35ns/row descriptor generation) can be used
    instead of generic indirect DMA (~8ns/row).
  - Groups are padded to a fixed capacity (input-independent kernel
    shape) and processed in chunks of CH edges: two dma_gathers (head,
    tail) + one strided rel load land [128, CH/128, 128] f32 tiles with
    edge i of the chunk at [i%128, i//128, :]; DVE does two elementwise
    multiplies + a blocked reduce over D=128 into a resident score
    plane, stored to HBM once at the end.
  - Host pre-permutes rel rows into the chunk layout and inverts the
    edge permutation on the returned score planes.
"""

import numpy as np

N_NODES = 100000
E_TOTAL = 600000
D = 128
N_CORES = 8
E_CORE = E_TOTAL // N_CORES  # 75000

NB = 4                # node blocks
BS = N_NODES // NB    # block size (rows per gather window)
G = NB * NB           # groups per core
CH = 2560             # edge slots per chunk (20 cols of 128)
CAP = 5120            # slots per group (must be multiple of CH)
S = G * CAP           # total slots per core
COLS = S // 128       # score plane columns

_CACHE: dict = {}


def _build_module(repeats: int = 1):
    import concourse.bacc as bacc
    import concourse.mybir as mybir
    from concourse.tile import TileContext

    nc = bacc.Bacc(
        "TRN2",
        debug=False,
        enable_asserts=False,
        target_bir_lowering=False,
        num_devices=N_CORES,
    )
    f32 = mybir.dt.float32
    i16 = mybir.dt.int16

    node = nc.dram_tensor("node_emb", [N_NODES, D], f32, kind="ExternalInput").ap()
    relsw = nc.dram_tensor("relsw", [128, S], f32, kind="ExternalInput").ap()
    srci = nc.dram_tensor("srci", [128, S // 16], i16, kind="ExternalInput").ap()
    dsti = nc.dram_tensor("dsti", [128, S // 16], i16, kind="ExternalInput").ap()
    out = nc.dram_tensor("scores", [128, COLS], f32, kind="ExternalOutput").ap()

    n_chunks = CAP // CH

    with TileContext(nc) as tc:
        with (
            tc.tile_pool(name="idx", bufs=1) as idxp,
            tc.tile_pool(name="big", bufs=5) as bigp,
            tc.tile_pool(name="res", bufs=1) as resp,
        ):
            src_t = idxp.tile([128, S // 16], i16, tag="srci")
            dst_t = idxp.tile([128, S // 16], i16, tag="dsti")
            score_t = resp.tile([128, COLS], f32, tag="score")
            nc.sync.dma_start(out=src_t[:], in_=srci[:])
            nc.sync.dma_start(out=dst_t[:], in_=dsti[:])

            for _rep in range(repeats):
              for g in range(G):
                sb = (g // NB) * BS
                db = (g % NB) * BS
                for c in range(n_chunks):
                    s0 = g * CAP + c * CH
                    head = bigp.tile([128, CH], f32, tag="head")
                    tail = bigp.tile([128, CH], f32, tag="tail")
                    relt = bigp.tile([128, CH], f32, tag="rel")
                    nc.gpsimd.dma_gather(
                        out_ap=head[:].rearrange("p (c d) -> p c d", d=D),
                        in_ap=node[sb : sb + BS],
                        idxs_ap=src_t[:, s0 // 16 : (s0 + CH) // 16],
                        num_idxs=CH,
                        num_idxs_reg=CH,
                        elem_size=D,
                        single_packet=False,
                    )
                    nc.gpsimd.dma_gather(
                        out_ap=tail[:].rearrange("p (c d) -> p c d", d=D),
                        in_ap=node[db : db + BS],
                        idxs_ap=dst_t[:, s0 // 16 : (s0 + CH) // 16],
                        num_idxs=CH,
                        num_idxs_reg=CH,
                        elem_size=D,
                        single_packet=False,
                    )
                    nc.sync.dma_start(out=relt[:], in_=relsw[:, s0 : s0 + CH])
                    nc.vector.tensor_tensor(
                        out=head[:], in0=head[:], in1=relt[:],
                        op=mybir.AluOpType.mult,
                    )
                    nc.vector.tensor_tensor(
                        out=head[:], in0=head[:], in1=tail[:],
                        op=mybir.AluOpType.mult,
                    )
                    nc.vector.tensor_reduce(
                        out=score_t[:, s0 // 128 : (s0 + CH) // 128],
                        in_=head[:].rearrange("p (c d) -> p c d", d=D),
                        axis=mybir.AxisListType.X,
                        op=mybir.AluOpType.add,
                    )

            nc.sync.dma_start(out=out[:], in_=score_t[:])

    nc.compile()
    return nc


def _get_module(repeats: int = 1):
    key = ("nc", repeats)
    if key not in _CACHE:
        _CACHE[key] = _build_module(repeats)
    return _CACHE[key]


def _wrap16(x: np.ndarray) -> np.ndarray:
    """[S] int16 -> [128, S/16] gather index plane (16-wrap, replicated 8x)."""
    w = x.reshape(S // 16, 16).T
    return np.ascontiguousarray(np.tile(w, (8, 1)))


def _prep_core(rel_c, src_c, dst_c):
    src_c = src_c.astype(np.int64)
    dst_c = dst_c.astype(np.int64)
    g = (src_c // BS) * NB + (dst_c // BS)
    order = np.argsort(g, kind="stable")
    gs = g[order]
    counts = np.bincount(g, minlength=G)
    if counts.max() > CAP:
        raise ValueError(f"group overflow: {counts.max()} > CAP={CAP}")
    cum = np.zeros(G, dtype=np.int64)
    cum[1:] = np.cumsum(counts)[:-1]
    rank = np.arange(E_CORE) - cum[gs]
    slots = gs * CAP + rank  # slot for each sorted edge

    loc_src = np.zeros(S, dtype=np.int16)
    loc_dst = np.zeros(S, dtype=np.int16)
    loc_src[slots] = (src_c[order] - (gs // NB) * BS).astype(np.int16)
    loc_dst[slots] = (dst_c[order] - (gs % NB) * BS).astype(np.int16)

    rel_perm = np.zeros((S, D), dtype=np.float32)
    rel_perm[slots] = rel_c[order]
    relsw = np.ascontiguousarray(
        rel_perm.reshape(S // 128, 128, D).transpose(1, 0, 2).reshape(128, S)
    )
    return (
        {"relsw": relsw, "srci": _wrap16(loc_src), "dsti": _wrap16(loc_dst)},
        order,
        slots,
    )


def make_in_maps(node_emb, rel_emb, src, dst):
    node = np.ascontiguousarray(np.asarray(node_emb, dtype=np.float32))
    rel_emb = np.asarray(rel_emb, dtype=np.float32)
    src = np.asarray(src)
    dst = np.asarray(dst)
    in_maps, metas = [], []
    for c in range(N_CORES):
        sl = slice(c * E_CORE, (c + 1) * E_CORE)
        m, order, slots = _prep_core(rel_emb[sl], src[sl], dst[sl])
        m["node_emb"] = node
        in_maps.append(m)
        metas.append((order, slots))
    return in_maps, metas


def gather_outputs(results, metas) -> np.ndarray:
    scores = np.empty(E_TOTAL, dtype=np.float32)
    for c in range(N_CORES):
        plane = np.asarray(results[c]["scores"], dtype=np.float32)  # [128, COLS]
        lin = plane.T.ravel()  # lin[slot], slot = col*128 + p
        order, slots = metas[c]
        out_c = np.empty(E_CORE, dtype=np.float32)
        out_c[order] = lin[slots]
        scores[c * E_CORE : (c + 1) * E_CORE] = out_c
    return scores


def run(node_emb, rel_emb, src, dst, trace=False):
    from concourse import bass_utils
    from concourse.bass_interp import get_hw_module

    nc = _get_module()
    in_maps, metas = make_in_maps(node_emb, rel_emb, src, dst)
    old_m = nc.m
    nc.m = get_hw_module(nc.m)
    try:
        res = bass_utils.run_bass_kernel_spmd(
            nc, in_maps, core_ids=list(range(N_CORES)), trace=trace
        )
    finally:
        nc.m = old_m
    return gather_outputs(res.results, metas), res


def kernel(node_emb, rel_emb, src, dst):
    scores, _ = run(node_emb, rel_emb, src, dst, trace=False)
    return scores



# revision 5
# speedup vs baseline: 4.2264x; 4.2264x over previous
"""DistMult edge scoring on 8 Trainium2 NeuronCores — v3.

score[e] = sum_d node_emb[src[e], d] * rel_emb[e, d] * node_emb[dst[e], d]

v3 over v2: edges are dealt round-robin to cores within each
(src_block, dst_block) group, with each group padded to a multiple of 8
by <=7 dummy edges. Every core then holds exactly N_g = padded_count/8
edges of group g, so the gather's num_idxs is a static per-group
constant: no padded-slot descriptors (v2 wasted ~9% of gather DMA on
pad slots), and cores are perfectly load-balanced. The per-group counts
are baked into the compiled module (module cache keyed by them).
"""

import numpy as np
import ml_dtypes

N_NODES = 100000
E_TOTAL = 600000
D = 128
N_CORES = 8

NB = 4                # node blocks
BS = N_NODES // NB    # block size (rows per gather window)
G = NB * NB           # groups per core
CH = 2560             # max edge slots per chunk (20 cols of 128)
CAP = 5120            # slot capacity per group (chunk-grid pitch)
S = G * CAP           # total slots per core
COLS = S // 128       # score plane columns

N_QUEUES = 4
N_BUFS = 10

_CACHE: dict = {}


def _build_module(counts: tuple, repeats: int = 1):
    """counts[g] = per-core edge count of group g (same on every core)."""
    import concourse.bacc as bacc
    import concourse.mybir as mybir
    from concourse.tile import TileContext

    assert len(counts) == G and all(0 < n <= CAP for n in counts)

    nc = bacc.Bacc(
        "TRN2",
        debug=False,
        enable_asserts=False,
        target_bir_lowering=False,
        num_devices=N_CORES,
        num_swdge_queues=N_QUEUES,
    )
    f32 = mybir.dt.float32
    bf16 = mybir.dt.bfloat16
    i16 = mybir.dt.int16

    node = nc.dram_tensor("node_emb", [N_NODES, D], bf16, kind="ExternalInput").ap()
    relsw = nc.dram_tensor("relsw", [128, S], bf16, kind="ExternalInput").ap()
    srci = nc.dram_tensor("srci", [128, S // 16], i16, kind="ExternalInput").ap()
    dsti = nc.dram_tensor("dsti", [128, S // 16], i16, kind="ExternalInput").ap()
    out = nc.dram_tensor("scores", [128, COLS], f32, kind="ExternalOutput").ap()

    with TileContext(nc) as tc:
        with (
            tc.tile_pool(name="idx", bufs=1) as idxp,
            tc.tile_pool(name="big", bufs=N_BUFS) as bigp,
            tc.tile_pool(name="res", bufs=1) as resp,
        ):
            src_t = idxp.tile([128, S // 16], i16, tag="srci")
            dst_t = idxp.tile([128, S // 16], i16, tag="dsti")
            score_t = resp.tile([128, COLS], f32, tag="score")
            nc.sync.dma_start(out=src_t[:], in_=srci[:])
            nc.sync.dma_start(out=dst_t[:], in_=dsti[:])
            nc.vector.memset(score_t[:], 0.0)  # pad slots are never reduced into

            qi = 0
            for _rep in range(repeats):
              for g in range(G):
                sb = (g // NB) * BS
                db = (g % NB) * BS
                n_left = counts[g]
                c = 0
                while n_left > 0:
                    n = min(n_left, CH)
                    n_left -= n
                    s0 = g * CAP + c * CH
                    c += 1
                    kb = (n + 127) // 128      # col blocks used
                    k16 = (n + 15) // 16       # idx columns used
                    head = bigp.tile([128, CH], bf16, tag="head")
                    tail = bigp.tile([128, CH], bf16, tag="tail")
                    relt = bigp.tile([128, CH], bf16, tag="rel")
                    nc.gpsimd.dma_gather(
                        out_ap=head[:, : kb * D].rearrange("p (c d) -> p c d", d=D),
                        in_ap=node[sb : sb + BS],
                        idxs_ap=src_t[:, s0 // 16 : s0 // 16 + k16],
                        num_idxs=n,
                        num_idxs_reg=n,
                        elem_size=D,
                        single_packet=False,
                        queue_num=qi % N_QUEUES,
                    )
                    qi += 1
                    nc.gpsimd.dma_gather(
                        out_ap=tail[:, : kb * D].rearrange("p (c d) -> p c d", d=D),
                        in_ap=node[db : db + BS],
                        idxs_ap=dst_t[:, s0 // 16 : s0 // 16 + k16],
                        num_idxs=n,
                        num_idxs_reg=n,
                        elem_size=D,
                        single_packet=False,
                        queue_num=qi % N_QUEUES,
                    )
                    qi += 1
                    nc.sync.dma_start(
                        out=relt[:, : kb * D], in_=relsw[:, s0 : s0 + kb * D]
                    )
                    nc.vector.tensor_tensor(
                        out=head[:, : kb * D], in0=head[:, : kb * D],
                        in1=relt[:, : kb * D], op=mybir.AluOpType.mult,
                    )
                    nc.vector.tensor_tensor(
                        out=head[:, : kb * D], in0=head[:, : kb * D],
                        in1=tail[:, : kb * D], op=mybir.AluOpType.mult,
                    )
                    nc.vector.tensor_reduce(
                        out=score_t[:, s0 // 128 : s0 // 128 + kb],
                        in_=head[:, : kb * D].rearrange("p (c d) -> p c d", d=D),
                        axis=mybir.AxisListType.X,
                        op=mybir.AluOpType.add,
                    )

            nc.sync.dma_start(out=out[:], in_=score_t[:])

    # The Tile scheduler reorders instructions and assigns SWDGE completion
    # semaphores round-robin (DMASW lanes, 8 of them) in FINAL stream order,
    # ignoring queue_num. A semaphore lane fed from two different SWDGE queues
    # breaks the scheduler's per-queue FIFO assumption (data race / potential
    # hang). Rewrite queue_num post-scheduling so lane i%8 always maps to
    # queue (i%8)%4 — consistent by construction.
    gi = 0
    for f in nc.m.functions:
        for b in f.blocks:
            for inst in b.instructions:
                if type(inst).__name__ == "InstDMAGatherAnt":
                    inst.queue_num = (gi % 8) % 4
                    gi += 1

    nc.compile()
    return nc


def _get_module(counts: tuple, repeats: int = 1):
    key = (counts, repeats)
    if key not in _CACHE:
        _CACHE[key] = _build_module(counts, repeats)
    return _CACHE[key]


def _wrap16(x: np.ndarray) -> np.ndarray:
    """[S] int16 -> [128, S/16] gather index plane (16-wrap, replicated 8x)."""
    w = x.reshape(S // 16, 16).T
    return np.ascontiguousarray(np.tile(w, (8, 1)))


def make_in_maps(node_emb, rel_emb, src, dst):
    """Deal edges round-robin to cores within each group; pad each group to a
    multiple of 8 with dummy edges (idx 0, rel 0).

    Returns (in_maps, metas, counts): counts[g] = per-core slots of group g;
    metas[c] = (edge_ids, slots) with global edge indices for core c.
    """
    node = np.ascontiguousarray(
        np.asarray(node_emb, dtype=np.float32).astype(ml_dtypes.bfloat16)
    )
    rel_emb = np.asarray(rel_emb, dtype=np.float32).astype(ml_dtypes.bfloat16)
    src = np.asarray(src).astype(np.int64)
    dst = np.asarray(dst).astype(np.int64)

    g_all = (src // BS) * NB + (dst // BS)          # [E_TOTAL]
    order_all = np.argsort(g_all, kind="stable")     # edges sorted by group
    g_sorted = g_all[order_all]
    counts_all = np.bincount(g_all, minlength=G)
    cum = np.zeros(G, dtype=np.int64)
    cum[1:] = np.cumsum(counts_all)[:-1]
    rank = np.arange(E_TOTAL) - cum[g_sorted]        # rank within group
    core_of = rank % N_CORES
    slot_of = g_sorted * CAP + rank // N_CORES

    counts = tuple(int(-(-c // N_CORES)) for c in counts_all)  # ceil(c/8)
    if max(counts) > CAP:
        raise ValueError(f"group overflow: {max(counts)} > CAP={CAP}")

    loc_src_val = (src[order_all] - (g_sorted // NB) * BS).astype(np.int16)
    loc_dst_val = (dst[order_all] - (g_sorted % NB) * BS).astype(np.int16)

    in_maps, metas = [], []
    for c in range(N_CORES):
        sel = core_of == c
        slots = slot_of[sel]
        edge_ids = order_all[sel]

        loc_src = np.zeros(S, dtype=np.int16)
        loc_dst = np.zeros(S, dtype=np.int16)
        loc_src[slots] = loc_src_val[sel]
        loc_dst[slots] = loc_dst_val[sel]

        rel_perm = np.zeros((S, D), dtype=ml_dtypes.bfloat16)
        rel_perm[slots] = rel_emb[edge_ids]
        relsw = np.ascontiguousarray(
            rel_perm.reshape(S // 128, 128, D).transpose(1, 0, 2).reshape(128, S)
        )
        in_maps.append(
            {
                "relsw": relsw,
                "srci": _wrap16(loc_src),
                "dsti": _wrap16(loc_dst),
                "node_emb": node,
            }
        )
        metas.append((edge_ids, slots))
    return in_maps, metas, counts


def gather_outputs(results, metas) -> np.ndarray:
    scores = np.empty(E_TOTAL, dtype=np.float32)
    for c in range(N_CORES):
        plane = np.asarray(results[c]["scores"], dtype=np.float32)  # [128, COLS]
        lin = plane.T.ravel()  # lin[slot], slot = col*128 + p
        edge_ids, slots = metas[c]
        scores[edge_ids] = lin[slots]
    return scores


def run(node_emb, rel_emb, src, dst, trace=False):
    from concourse import bass_utils
    from concourse.bass_interp import get_hw_module

    in_maps, metas, counts = make_in_maps(node_emb, rel_emb, src, dst)
    nc = _get_module(counts)
    old_m = nc.m
    nc.m = get_hw_module(nc.m)
    try:
        res = bass_utils.run_bass_kernel_spmd(
            nc, in_maps, core_ids=list(range(N_CORES)), trace=trace
        )
    finally:
        nc.m = old_m
    return gather_outputs(res.results, metas), res


def kernel(node_emb, rel_emb, src, dst):
    scores, _ = run(node_emb, rel_emb, src, dst, trace=False)
    return scores
